# revision 11
# baseline (speedup 1.0000x reference)
"""Trainium2 Bass kernel for nn_CMB_H_OMBH2 (MLP -> natural cubic spline -> grid eval).

Strategy:
  - 8 NeuronCores, data-parallel over grid rows: core c evaluates grid rows
    [32c, 32c+32) for all 256 channels.
  - MLP + spline setup (tiny) replicated on every core.
  - Tridiagonal spline solve via Newton-Schulz inverse on the tensor engine
    (A is SPD diagonally dominant: 8 iterations reach fp32 accuracy).
  - Spline evaluation reformulated in a clamped truncated-power basis:
        val(x) = a0 + sum_j [ d_j*C_j(x) + (M_j/2)*S_j(x) + b_j*L_j(x) ]
    where L_j = clip(x - kn_j, 0, h_j), S_j = L_j^2, C_j = L_j^3 (last knot
    unclamped).  This is exact (spline-coefficient continuity) and well
    conditioned, and turns searchsorted+gather+Horner into 3 dense matmuls
    (float32r) over a basis built with one fp32 matmul broadcast + Relu +
    clamp + two multiplies.
"""
import sys
import numpy as np

sys.path.insert(0, "/opt/trn_rl_repo")

N_CORES = 8
ROWS_PER_CORE = 32          # grid rows per core
PTS = ROWS_PER_CORE * 256   # 8192 points per core
CHUNK = 512                 # psum-bank sized eval chunk
SUPER = 2048                # sbuf supertile width
THETA_LO = (50.0, 0.0075)
THETA_SCALE = (40.0, 0.0492)
BIG = 3.0e38

_CACHE = {}


def _build_program():
    import concourse.bacc as bacc
    import concourse.tile as tile
    import concourse.mybir as mybir

    dt = mybir.dt
    Alu = mybir.AluOpType
    Act = mybir.ActivationFunctionType

    nc = bacc.Bacc("TRN2", target_bir_lowering=False, debug=False,
                   num_devices=N_CORES)

    f32 = dt.float32
    f32r = dt.float32r

    theta = nc.dram_tensor("theta", [256, 2], f32, kind="ExternalInput").ap()
    W0 = nc.dram_tensor("W0", [2, 100], f32, kind="ExternalInput").ap()
    b0 = nc.dram_tensor("b0", [100], f32, kind="ExternalInput").ap()
    W1 = nc.dram_tensor("W1", [100, 100], f32, kind="ExternalInput").ap()
    b1 = nc.dram_tensor("b1", [100], f32, kind="ExternalInput").ap()
    W2 = nc.dram_tensor("W2", [100, 100], f32, kind="ExternalInput").ap()
    b2 = nc.dram_tensor("b2", [100], f32, kind="ExternalInput").ap()
    W3 = nc.dram_tensor("W3", [100, 128], f32, kind="ExternalInput").ap()
    b3 = nc.dram_tensor("b3", [128], f32, kind="ExternalInput").ap()
    knots = nc.dram_tensor("knots", [128], f32, kind="ExternalInput").ap()
    gslice = nc.dram_tensor("gslice", [ROWS_PER_CORE, 256], f32,
                            kind="ExternalInput").ap()
    out_d = nc.dram_tensor("out", [256, ROWS_PER_CORE, 256], f32,
                           kind="ExternalOutput").ap()

    with tile.TileContext(nc) as tc:
        with (
            tc.tile_pool(name="const", bufs=1) as cpool,
            tc.tile_pool(name="work", bufs=1) as wpool,
            tc.tile_pool(name="newton", bufs=2) as npool,
            tc.tile_pool(name="zps", bufs=2, space="PSUM") as zpsum,
            tc.tile_pool(name="vps", bufs=4, space="PSUM") as vpsum,
            tc.tile_pool(name="sps", bufs=2, space="PSUM") as spsum,
            tc.tile_pool(name="sup", bufs=3) as spool,
            tc.tile_pool(name="outp", bufs=6) as opool,
        ):
            # ---------------- load small inputs ----------------
            thetaT = cpool.tile([2, 256], f32)
            nc.sync.dma_start(thetaT[:], theta.rearrange("b k -> k b"))
            w0sb = cpool.tile([2, 100], f32)
            nc.sync.dma_start(w0sb[:], W0[:])
            w1sb = cpool.tile([100, 100], f32)
            nc.sync.dma_start(w1sb[:], W1[:])
            w2sb = cpool.tile([100, 100], f32)
            nc.sync.dma_start(w2sb[:], W2[:])
            w3sb = cpool.tile([100, 128], f32)
            nc.sync.dma_start(w3sb[:], W3[:])
            b0c = cpool.tile([100, 1], f32)
            nc.sync.dma_start(b0c[:], b0.rearrange("(p o) -> p o", o=1))
            b1c = cpool.tile([100, 1], f32)
            nc.sync.dma_start(b1c[:], b1.rearrange("(p o) -> p o", o=1))
            b2c = cpool.tile([100, 1], f32)
            nc.sync.dma_start(b2c[:], b2.rearrange("(p o) -> p o", o=1))
            b3c = cpool.tile([128, 1], f32)
            nc.sync.dma_start(b3c[:], b3.rearrange("(p o) -> p o", o=1))
            knr = cpool.tile([1, 128], f32)
            nc.sync.dma_start(knr[:], knots.rearrange("(o k) -> o k", o=1))
            # x row (this core's 8192 grid values, natural order)
            xr = cpool.tile([2, PTS], f32)
            nc.vector.memset(xr[:], 1.0)
            nc.sync.dma_start(
                xr[0:1, :], gslice.rearrange("a b -> (a b)").rearrange("(o k) -> o k", o=1))

            # ---------------- MLP (transposed activations) ----------------
            lr = cpool.tile([1, 4], f32)
            nc.vector.memset(lr[:, 0:1], float(THETA_LO[0]))
            nc.vector.memset(lr[:, 1:2], float(THETA_LO[1]))
            nc.vector.memset(lr[:, 2:3], float(1.0 / np.float32(THETA_SCALE[0])))
            nc.vector.memset(lr[:, 3:4], float(1.0 / np.float32(THETA_SCALE[1])))
            lo_c = cpool.tile([2, 1], f32)
            nc.sync.dma_start(lo_c[:], lr[:, 0:2])
            isc_c = cpool.tile([2, 1], f32)
            nc.sync.dma_start(isc_c[:], lr[:, 2:4])
            tn = cpool.tile([2, 256], f32)
            nc.vector.tensor_scalar(tn[:], thetaT[:], lo_c[:], isc_c[:],
                                    Alu.subtract, Alu.mult)

            hp = spsum.tile([100, 256], f32, tag="sp")
            nc.tensor.matmul(hp[:], w0sb[:], tn[:], start=True, stop=True)
            h0t = cpool.tile([100, 256], f32)
            nc.scalar.activation(h0t[:], hp[:], Act.Relu, bias=b0c[:])
            hp1 = spsum.tile([100, 256], f32, tag="sp")
            nc.tensor.matmul(hp1[:], w1sb[:], h0t[:], start=True, stop=True)
            h1t = cpool.tile([100, 256], f32)
            nc.scalar.activation(h1t[:], hp1[:], Act.Relu, bias=b1c[:])
            hp2 = spsum.tile([100, 256], f32, tag="sp")
            nc.tensor.matmul(hp2[:], w2sb[:], h1t[:], start=True, stop=True)
            h2t = cpool.tile([100, 256], f32)
            nc.scalar.activation(h2t[:], hp2[:], Act.Relu, bias=b2c[:])
            hp3 = spsum.tile([128, 256], f32, tag="sp")
            nc.tensor.matmul(hp3[:], w3sb[:], h2t[:], start=True, stop=True)
            outT = cpool.tile([128, 256], f32)   # outT[m, b] = out[b, m]
            nc.scalar.activation(outT[:], hp3[:], Act.Identity, bias=b3c[:])

            # ---------------- reshape: y[i, j] = out[2i + (j>=128), j%128] --------
            ident = cpool.tile([128, 128], f32)
            ones_col = cpool.tile([128, 1], f32)
            nc.vector.memset(ones_col[:], 1.0)
            nc.gpsimd.affine_select(ident[:], ones_col[:].broadcast_to([128, 128]),
                                    pattern=[[-1, 128]], base=0,
                                    channel_multiplier=1,
                                    compare_op=Alu.is_equal, fill=0.0)
            outT3 = outT[:].rearrange("m (b t) -> m t b", t=2)
            y_t = cpool.tile([128, 256], f32)
            tp = spsum.tile([128, 128], f32, tag="sp")
            nc.tensor.transpose(tp[:], outT3[:, 0, :], ident[:])
            nc.scalar.copy(y_t[:, 0:128], tp[:])
            tp1 = spsum.tile([128, 128], f32, tag="sp")
            nc.tensor.transpose(tp1[:], outT3[:, 1, :], ident[:])
            nc.scalar.copy(y_t[:, 128:256], tp1[:])

            # ---------------- spline solve (Newton-Schulz) ----------------
            # per-knot scalar vectors built on the free axis (partition 0),
            # then DMA-transposed into columns of `cols`
            rw = cpool.tile([1, 8 * 128], f32)
            rwv = rw[:].rearrange("o (r k) -> o r k", r=8)
            nc.vector.memset(rw[:], 0.0)
            # r0: h_j = kn[j+1]-kn[j] (j<127)
            nc.vector.tensor_tensor(rwv[:, 0, 0:127], knr[:, 1:128], knr[:, 0:127],
                                    Alu.subtract)
            # r1: h_{j+1} (j<126)
            nc.vector.tensor_copy(rwv[:, 1, 0:126], rwv[:, 0, 1:127])
            # r2: dg = 2*(h_j + h_{j+1}) (j<126)
            nc.vector.tensor_tensor(rwv[:, 2, 0:126], rwv[:, 0, 0:126],
                                    rwv[:, 1, 0:126], Alu.add)
            nc.vector.tensor_scalar_mul(rwv[:, 2, 0:126], rwv[:, 2, 0:126], 2.0)
            # r3: 1/dg
            nc.vector.reciprocal(rwv[:, 3, 0:126], rwv[:, 2, 0:126])
            # r4: 1/h
            nc.vector.reciprocal(rwv[:, 4, 0:127], rwv[:, 0, 0:127])
            # r5: 1/(6h);  r6: -h/6
            nc.vector.tensor_scalar_mul(rwv[:, 5, 0:127], rwv[:, 4, 0:127],
                                        float(1.0 / 6.0))
            nc.vector.tensor_scalar_mul(rwv[:, 6, 0:127], rwv[:, 0, 0:127],
                                        float(-1.0 / 6.0))
            # r7: caps = h_j (j<126), BIG, 0
            nc.vector.tensor_copy(rwv[:, 7, 0:126], rwv[:, 0, 0:126])
            nc.vector.memset(rwv[:, 7, 126:127], BIG)
            nc.vector.memset(rwv[:, 7, 127:128], 0.0)
            cols = cpool.tile([128, 8], f32)
            for r in range(8):
                nc.sync.dma_start(cols[:, r:r + 1], rwv[:, r, :])
            h_c = cols[:, 0:1]
            h1_c = cols[:, 1:2]
            dg_c = cols[:, 2:3]
            rd_c = cols[:, 3:4]
            rh_c = cols[:, 4:5]
            rh6_c = cols[:, 5:6]
            hneg6_c = cols[:, 6:7]
            caps_c = cols[:, 7:8]

            a_t = cpool.tile([126, 126], f32)
            a_u = wpool.tile([126, 126], f32)
            a_l = wpool.tile([126, 126], f32)
            nc.gpsimd.affine_select(a_t[:], dg_c[0:126, :].broadcast_to([126, 126]),
                                    pattern=[[-1, 126]], base=0, channel_multiplier=1,
                                    compare_op=Alu.is_equal, fill=0.0)
            nc.gpsimd.affine_select(a_u[:], h1_c[0:126, :].broadcast_to([126, 126]),
                                    pattern=[[-1, 126]], base=1, channel_multiplier=1,
                                    compare_op=Alu.is_equal, fill=0.0)
            nc.gpsimd.affine_select(a_l[:], h_c[0:126, :].broadcast_to([126, 126]),
                                    pattern=[[-1, 126]], base=-1, channel_multiplier=1,
                                    compare_op=Alu.is_equal, fill=0.0)
            nc.vector.tensor_tensor(a_t[:], a_t[:], a_u[:], Alu.add)
            nc.vector.tensor_tensor(a_t[:], a_t[:], a_l[:], Alu.add)

            i2 = cpool.tile([126, 126], f32)
            two_col = cpool.tile([126, 1], f32)
            nc.vector.memset(two_col[:], 2.0)
            nc.gpsimd.affine_select(i2[:], two_col[:].broadcast_to([126, 126]),
                                    pattern=[[-1, 126]], base=0, channel_multiplier=1,
                                    compare_op=Alu.is_equal, fill=0.0)

            x_cur = npool.tile([126, 126], f32, tag="xn")
            nc.gpsimd.affine_select(x_cur[:], rd_c[0:126, :].broadcast_to([126, 126]),
                                    pattern=[[-1, 126]], base=0, channel_multiplier=1,
                                    compare_op=Alu.is_equal, fill=0.0)
            for it in range(8):
                eps = spsum.tile([126, 126], f32, tag="sp")
                nc.tensor.matmul(eps[:], a_t[:], x_cur[:], start=True, stop=True)
                y_n = npool.tile([126, 126], f32, tag="yn")
                nc.vector.scalar_tensor_tensor(y_n[:], eps[:], -1.0, i2[:],
                                               Alu.mult, Alu.add)
                xps = spsum.tile([126, 126], f32, tag="sp")
                nc.tensor.matmul(xps[:], x_cur[:], y_n[:], start=True, stop=True)
                x_new = npool.tile([126, 126], f32, tag="xn")
                nc.scalar.copy(x_new[:], xps[:])
                x_cur = x_new
            x6 = wpool.tile([126, 126], f32)
            nc.vector.tensor_scalar_mul(x6[:], x_cur[:], 6.0)

            y_sh = wpool.tile([127, 256], f32)
            nc.sync.dma_start(y_sh[:], y_t[1:128, :])
            dy = wpool.tile([127, 256], f32)
            nc.vector.tensor_tensor(dy[:], y_sh[:], y_t[0:127, :], Alu.subtract)
            s_sl = wpool.tile([127, 256], f32)
            nc.vector.tensor_scalar_mul(s_sl[:], dy[:], rh_c[0:127, :])
            s_sh = wpool.tile([126, 256], f32)
            nc.sync.dma_start(s_sh[:], s_sl[1:127, :])
            rhs_i = wpool.tile([126, 256], f32)
            nc.vector.tensor_tensor(rhs_i[:], s_sh[:], s_sl[0:126, :],
                                    Alu.subtract)
            mps = spsum.tile([126, 256], f32, tag="sp")
            nc.tensor.matmul(mps[:], x6[:], rhs_i[:], start=True, stop=True)
            m_in = wpool.tile([126, 256], f32)
            nc.scalar.copy(m_in[:], mps[:])
            m_t = wpool.tile([128, 256], f32)
            nc.vector.memset(m_t[:], 0.0)
            nc.sync.dma_start(m_t[1:127, :], m_in[:])
            m_sh = wpool.tile([127, 256], f32)
            nc.vector.memset(m_sh[:], 0.0)
            nc.sync.dma_start(m_sh[0:126, :], m_in[:])

            # ---------------- basis weights (f32r) ----------------
            # W3w = d_j = (M[j+1]-M[j]) / (6 h_j); W2w = M[j]/2; W1w = b_j
            dm = wpool.tile([127, 256], f32)
            nc.vector.tensor_tensor(dm[:], m_sh[:], m_t[0:127, :], Alu.subtract)
            w3w = cpool.tile([127, 256], f32r)
            nc.vector.tensor_scalar_mul(w3w[:], dm[:], rh6_c[0:127, :])
            w2w = cpool.tile([127, 256], f32r)
            nc.vector.tensor_scalar_mul(w2w[:], m_t[0:127, :], 0.5)
            t1 = wpool.tile([127, 256], f32)
            nc.vector.scalar_tensor_tensor(t1[:], m_t[0:127, :], 2.0, m_sh[:],
                                           Alu.mult, Alu.add)
            w1w = cpool.tile([127, 256], f32r)
            nc.vector.scalar_tensor_tensor(w1w[:], t1[:], hneg6_c[0:127, :], s_sl[:],
                                           Alu.mult, Alu.add)

            # Z-matmul weights (fp32, exact): [ones; -kn]
            negkn = cpool.tile([1, 128], f32)
            nc.vector.tensor_scalar_mul(negkn[:], knr[:], -1.0)
            knw = cpool.tile([2, 128], f32)
            nc.vector.memset(knw[:], 1.0)
            nc.sync.dma_start(knw[1:2, :], negkn[:])

            # ---------------- evaluation ----------------
            n_chunks = PTS // CHUNK
            for ci in range(n_chunks):
                n0 = ci * CHUNK
                zp = zpsum.tile([128, CHUNK], f32)
                nc.tensor.matmul(zp[:], knw[:], xr[:, n0:n0 + CHUNK],
                                 start=True, stop=True)
                u_t = spool.tile([128, CHUNK], f32, tag="u")
                nc.scalar.activation(u_t[:], zp[:], Act.Relu)
                uc = spool.tile([128, CHUNK], f32r, tag="uc")
                nc.vector.tensor_scalar(uc[:], u_t[:], caps_c[:], None, Alu.min)
                s_t = spool.tile([128, CHUNK], f32r, tag="s")
                nc.vector.tensor_tensor(s_t[:], uc[:], uc[:], Alu.mult)
                p_t = spool.tile([128, CHUNK], f32r, tag="p")
                nc.vector.tensor_tensor(p_t[:], uc[:], s_t[:], Alu.mult)
                for half in range(2):
                    cs = slice(half * 128, (half + 1) * 128)
                    a0bias = outT[:, half:half + 1]
                    vp = vpsum.tile([128, CHUNK], f32)
                    nc.tensor.matmul(vp[:], w3w[:, cs], p_t[0:127, :],
                                     start=True, stop=False)
                    nc.tensor.matmul(vp[:], w2w[:, cs], s_t[0:127, :],
                                     start=False, stop=False)
                    nc.tensor.matmul(vp[:], w1w[:, cs], uc[0:127, :],
                                     start=False, stop=True)
                    ob = opool.tile([128, CHUNK], f32, tag="ob")
                    nc.scalar.activation(ob[:], vp[:], Act.Identity, bias=a0bias)
                    nc.sync.dma_start(out_d[cs, 2 * ci:2 * ci + 2, :], ob[:])
    nc.compile()
    return nc


def kernel(**inputs):
    from concourse.bass_utils import run_bass_kernel_spmd

    if "nc" not in _CACHE:
        _CACHE["nc"] = _build_program()
    nc = _CACHE["nc"]

    grid = np.ascontiguousarray(inputs["grid"], dtype=np.float32)
    common = {k: np.ascontiguousarray(np.asarray(v), dtype=np.float32)
              for k, v in inputs.items() if k != "grid"}
    in_maps = []
    for c in range(N_CORES):
        m = dict(common)
        m["gslice"] = np.ascontiguousarray(
            grid[c * ROWS_PER_CORE:(c + 1) * ROWS_PER_CORE])
        in_maps.append(m)
    res = run_bass_kernel_spmd(nc, in_maps, list(range(N_CORES)),
                               trace=bool(_CACHE.get("trace", False)),
                               tmpdir=_CACHE.get("tmpdir"))
    _CACHE["last_res"] = res
    out = np.concatenate([res.results[c]["out"] for c in range(N_CORES)], axis=1)
    return out


# revision 17
# speedup vs baseline: 1.0592x; 1.0592x over previous
"""Trainium2 Bass kernel for nn_CMB_H_OMBH2 (MLP -> natural cubic spline -> grid eval).

Strategy:
  - 8 NeuronCores, data-parallel over grid rows: core c evaluates grid rows
    [32c, 32c+32) for all 256 channels.
  - MLP + spline setup (tiny) replicated on every core.
  - Tridiagonal spline solve via Newton-Schulz inverse on the tensor engine
    (A is SPD diagonally dominant: 8 iterations reach fp32 accuracy).
  - Spline evaluation reformulated in a clamped truncated-power basis:
        val(x) = a0 + sum_j [ d_j*C_j(x) + (M_j/2)*S_j(x) + b_j*L_j(x) ]
    where L_j = clip(x - kn_j, 0, h_j), S_j = L_j^2, C_j = L_j^3 (last knot
    unclamped).  This is exact (spline-coefficient continuity) and well
    conditioned, and turns searchsorted+gather+Horner into 3 dense matmuls
    (float32r) over a basis built with one fp32 matmul broadcast + Relu +
    clamp + two multiplies.
"""
import sys
import numpy as np

sys.path.insert(0, "/opt/trn_rl_repo")

N_CORES = 8
ROWS_PER_CORE = 32          # grid rows per core
PTS = ROWS_PER_CORE * 256   # 8192 points per core
CHUNK = 512                 # psum-bank sized eval chunk
SUPER = 2048                # sbuf supertile width
THETA_LO = (50.0, 0.0075)
THETA_SCALE = (40.0, 0.0492)
BIG = 3.0e38

_CACHE = {}


def _build_program():
    import concourse.bacc as bacc
    import concourse.tile as tile
    import concourse.mybir as mybir

    dt = mybir.dt
    Alu = mybir.AluOpType
    Act = mybir.ActivationFunctionType

    nc = bacc.Bacc("TRN2", target_bir_lowering=False, debug=False,
                   num_devices=N_CORES)

    f32 = dt.float32
    f32r = dt.float32r

    theta = nc.dram_tensor("theta", [256, 2], f32, kind="ExternalInput").ap()
    W0 = nc.dram_tensor("W0", [2, 100], f32, kind="ExternalInput").ap()
    b0 = nc.dram_tensor("b0", [100], f32, kind="ExternalInput").ap()
    W1 = nc.dram_tensor("W1", [100, 100], f32, kind="ExternalInput").ap()
    b1 = nc.dram_tensor("b1", [100], f32, kind="ExternalInput").ap()
    W2 = nc.dram_tensor("W2", [100, 100], f32, kind="ExternalInput").ap()
    b2 = nc.dram_tensor("b2", [100], f32, kind="ExternalInput").ap()
    W3 = nc.dram_tensor("W3", [100, 128], f32, kind="ExternalInput").ap()
    b3 = nc.dram_tensor("b3", [128], f32, kind="ExternalInput").ap()
    knots = nc.dram_tensor("knots", [128], f32, kind="ExternalInput").ap()
    gslice = nc.dram_tensor("gslice", [ROWS_PER_CORE, 256], f32,
                            kind="ExternalInput").ap()
    out_d = nc.dram_tensor("out", [256, ROWS_PER_CORE, 256], f32,
                           kind="ExternalOutput").ap()

    with tile.TileContext(nc) as tc:
        with (
            tc.tile_pool(name="const", bufs=1) as cpool,
            tc.tile_pool(name="work", bufs=1) as wpool,
            tc.tile_pool(name="newton", bufs=2) as npool,
            tc.tile_pool(name="zps", bufs=2, space="PSUM") as zpsum,
            tc.tile_pool(name="vps", bufs=4, space="PSUM") as vpsum,
            tc.tile_pool(name="sps", bufs=2, space="PSUM") as spsum,
            tc.tile_pool(name="sup", bufs=3) as spool,
            tc.tile_pool(name="outp", bufs=6) as opool,
        ):
            # ---------------- load small inputs ----------------
            thetaT = cpool.tile([2, 256], f32)
            nc.sync.dma_start(thetaT[:], theta.rearrange("b k -> k b"))
            w0sb = cpool.tile([2, 100], f32)
            nc.sync.dma_start(w0sb[:], W0[:])
            w1sb = cpool.tile([100, 100], f32)
            nc.sync.dma_start(w1sb[:], W1[:])
            w2sb = cpool.tile([100, 100], f32)
            nc.sync.dma_start(w2sb[:], W2[:])
            w3sb = cpool.tile([100, 128], f32)
            nc.sync.dma_start(w3sb[:], W3[:])
            b0c = cpool.tile([100, 1], f32)
            nc.sync.dma_start(b0c[:], b0.rearrange("(p o) -> p o", o=1))
            b1c = cpool.tile([100, 1], f32)
            nc.sync.dma_start(b1c[:], b1.rearrange("(p o) -> p o", o=1))
            b2c = cpool.tile([100, 1], f32)
            nc.sync.dma_start(b2c[:], b2.rearrange("(p o) -> p o", o=1))
            b3c = cpool.tile([128, 1], f32)
            nc.sync.dma_start(b3c[:], b3.rearrange("(p o) -> p o", o=1))
            knr = cpool.tile([1, 128], f32)
            nc.sync.dma_start(knr[:], knots.rearrange("(o k) -> o k", o=1))
            # x row (this core's 8192 grid values, natural order)
            xr = cpool.tile([2, PTS], f32)
            nc.gpsimd.memset(xr[:], 1.0)
            nc.sync.dma_start(
                xr[0:1, :], gslice.rearrange("a b -> (a b)").rearrange("(o k) -> o k", o=1))

            # ---------------- MLP (transposed activations) ----------------
            lr = cpool.tile([1, 4], f32)
            nc.vector.memset(lr[:, 0:1], float(THETA_LO[0]))
            nc.vector.memset(lr[:, 1:2], float(THETA_LO[1]))
            nc.vector.memset(lr[:, 2:3], float(1.0 / np.float32(THETA_SCALE[0])))
            nc.vector.memset(lr[:, 3:4], float(1.0 / np.float32(THETA_SCALE[1])))
            lo_c = cpool.tile([2, 1], f32)
            nc.gpsimd.dma_start(lo_c[:], lr[:, 0:2])
            isc_c = cpool.tile([2, 1], f32)
            nc.gpsimd.dma_start(isc_c[:], lr[:, 2:4])
            tn = cpool.tile([2, 256], f32)
            nc.vector.tensor_scalar(tn[:], thetaT[:], lo_c[:], isc_c[:],
                                    Alu.subtract, Alu.mult)

            hp = spsum.tile([100, 256], f32, tag="sp")
            nc.tensor.matmul(hp[:], w0sb[:], tn[:], start=True, stop=True)
            h0t = cpool.tile([100, 256], f32)
            nc.scalar.activation(h0t[:], hp[:], Act.Relu, bias=b0c[:])
            hp1 = spsum.tile([100, 256], f32, tag="sp")
            nc.tensor.matmul(hp1[:], w1sb[:], h0t[:], start=True, stop=True)
            h1t = cpool.tile([100, 256], f32)
            nc.scalar.activation(h1t[:], hp1[:], Act.Relu, bias=b1c[:])
            hp2 = spsum.tile([100, 256], f32, tag="sp")
            nc.tensor.matmul(hp2[:], w2sb[:], h1t[:], start=True, stop=True)
            h2t = cpool.tile([100, 256], f32)
            nc.scalar.activation(h2t[:], hp2[:], Act.Relu, bias=b2c[:])
            hp3 = spsum.tile([128, 256], f32, tag="sp")
            nc.tensor.matmul(hp3[:], w3sb[:], h2t[:], start=True, stop=True)
            outT = cpool.tile([128, 256], f32)   # outT[m, b] = out[b, m]
            nc.scalar.activation(outT[:], hp3[:], Act.Identity, bias=b3c[:])

            # ---------------- reshape: y[i, j] = out[2i + (j>=128), j%128] --------
            ident = cpool.tile([128, 128], f32)
            ones_col = cpool.tile([128, 1], f32)
            nc.vector.memset(ones_col[:], 1.0)
            nc.gpsimd.affine_select(ident[:], ones_col[:].broadcast_to([128, 128]),
                                    pattern=[[-1, 128]], base=0,
                                    channel_multiplier=1,
                                    compare_op=Alu.is_equal, fill=0.0)
            outT3 = outT[:].rearrange("m (b t) -> m t b", t=2)
            y_t = cpool.tile([128, 256], f32)
            tp = spsum.tile([128, 128], f32, tag="sp")
            nc.tensor.transpose(tp[:], outT3[:, 0, :], ident[:])
            nc.scalar.copy(y_t[:, 0:128], tp[:])
            tp1 = spsum.tile([128, 128], f32, tag="sp")
            nc.tensor.transpose(tp1[:], outT3[:, 1, :], ident[:])
            nc.scalar.copy(y_t[:, 128:256], tp1[:])

            # ---------------- spline solve (Newton-Schulz) ----------------
            # per-knot scalar vectors built on the free axis (partition 0),
            # then DMA-transposed into columns of `cols`
            rw = cpool.tile([1, 8 * 128], f32)
            rwv = rw[:].rearrange("o (r k) -> o r k", r=8)
            nc.vector.memset(rw[:], 0.0)
            # r0: h_j = kn[j+1]-kn[j] (j<127)
            nc.vector.tensor_tensor(rwv[:, 0, 0:127], knr[:, 1:128], knr[:, 0:127],
                                    Alu.subtract)
            # r1: h_{j+1} (j<126)
            nc.vector.tensor_copy(rwv[:, 1, 0:126], rwv[:, 0, 1:127])
            # r2: dg = 2*(h_j + h_{j+1}) (j<126)
            nc.vector.tensor_tensor(rwv[:, 2, 0:126], rwv[:, 0, 0:126],
                                    rwv[:, 1, 0:126], Alu.add)
            nc.vector.tensor_scalar_mul(rwv[:, 2, 0:126], rwv[:, 2, 0:126], 2.0)
            # r3: 1/dg
            nc.vector.reciprocal(rwv[:, 3, 0:126], rwv[:, 2, 0:126])
            # r4: 1/h
            nc.vector.reciprocal(rwv[:, 4, 0:127], rwv[:, 0, 0:127])
            # r5: 1/(6h);  r6: -h/6
            nc.vector.tensor_scalar_mul(rwv[:, 5, 0:127], rwv[:, 4, 0:127],
                                        float(1.0 / 6.0))
            nc.vector.tensor_scalar_mul(rwv[:, 6, 0:127], rwv[:, 0, 0:127],
                                        float(-1.0 / 6.0))
            # r7: caps = h_j (j<126), BIG, 0
            nc.vector.tensor_copy(rwv[:, 7, 0:126], rwv[:, 0, 0:126])
            nc.vector.memset(rwv[:, 7, 126:127], BIG)
            nc.vector.memset(rwv[:, 7, 127:128], 0.0)
            cols = cpool.tile([128, 8], f32)
            for r in range(8):
                nc.gpsimd.dma_start(cols[:, r:r + 1], rwv[:, r, :])
            h_c = cols[:, 0:1]
            h1_c = cols[:, 1:2]
            dg_c = cols[:, 2:3]
            rd_c = cols[:, 3:4]
            rh_c = cols[:, 4:5]
            rh6_c = cols[:, 5:6]
            hneg6_c = cols[:, 6:7]
            caps_c = cols[:, 7:8]

            a_t = cpool.tile([126, 126], f32)
            a_u = wpool.tile([126, 126], f32)
            a_l = wpool.tile([126, 126], f32)
            nc.gpsimd.affine_select(a_t[:], dg_c[0:126, :].broadcast_to([126, 126]),
                                    pattern=[[-1, 126]], base=0, channel_multiplier=1,
                                    compare_op=Alu.is_equal, fill=0.0)
            nc.gpsimd.affine_select(a_u[:], h1_c[0:126, :].broadcast_to([126, 126]),
                                    pattern=[[-1, 126]], base=1, channel_multiplier=1,
                                    compare_op=Alu.is_equal, fill=0.0)
            nc.gpsimd.affine_select(a_l[:], h_c[0:126, :].broadcast_to([126, 126]),
                                    pattern=[[-1, 126]], base=-1, channel_multiplier=1,
                                    compare_op=Alu.is_equal, fill=0.0)
            nc.vector.tensor_tensor(a_t[:], a_t[:], a_u[:], Alu.add)
            nc.vector.tensor_tensor(a_t[:], a_t[:], a_l[:], Alu.add)

            i2 = cpool.tile([126, 126], f32)
            two_col = cpool.tile([126, 1], f32)
            nc.vector.memset(two_col[:], 2.0)
            nc.gpsimd.affine_select(i2[:], two_col[:].broadcast_to([126, 126]),
                                    pattern=[[-1, 126]], base=0, channel_multiplier=1,
                                    compare_op=Alu.is_equal, fill=0.0)

            x_cur = npool.tile([126, 126], f32, tag="xn")
            nc.gpsimd.affine_select(x_cur[:], rd_c[0:126, :].broadcast_to([126, 126]),
                                    pattern=[[-1, 126]], base=0, channel_multiplier=1,
                                    compare_op=Alu.is_equal, fill=0.0)
            for it in range(6):
                eps = spsum.tile([126, 126], f32, tag="sp")
                nc.tensor.matmul(eps[:], a_t[:], x_cur[:], start=True, stop=True)
                y_n = npool.tile([126, 126], f32, tag="yn")
                nc.vector.scalar_tensor_tensor(y_n[:], eps[:], -1.0, i2[:],
                                               Alu.mult, Alu.add)
                xps = spsum.tile([126, 126], f32, tag="sp")
                nc.tensor.matmul(xps[:], x_cur[:], y_n[:], start=True, stop=True)
                x_new = npool.tile([126, 126], f32, tag="xn")
                nc.scalar.copy(x_new[:], xps[:])
                x_cur = x_new
            x6 = wpool.tile([126, 126], f32)
            nc.vector.tensor_scalar_mul(x6[:], x_cur[:], 6.0)

            y_sh = wpool.tile([127, 256], f32)
            nc.gpsimd.dma_start(y_sh[:], y_t[1:128, :])
            dy = wpool.tile([127, 256], f32)
            nc.vector.tensor_tensor(dy[:], y_sh[:], y_t[0:127, :], Alu.subtract)
            s_sl = wpool.tile([127, 256], f32)
            nc.vector.tensor_scalar_mul(s_sl[:], dy[:], rh_c[0:127, :])
            s_sh = wpool.tile([126, 256], f32)
            nc.gpsimd.dma_start(s_sh[:], s_sl[1:127, :])
            rhs_i = wpool.tile([126, 256], f32)
            nc.vector.tensor_tensor(rhs_i[:], s_sh[:], s_sl[0:126, :],
                                    Alu.subtract)
            mps = spsum.tile([126, 256], f32, tag="sp")
            nc.tensor.matmul(mps[:], x6[:], rhs_i[:], start=True, stop=True)
            m_in = wpool.tile([126, 256], f32)
            nc.scalar.copy(m_in[:], mps[:])
            m_t = wpool.tile([128, 256], f32)
            nc.vector.memset(m_t[:], 0.0)
            nc.gpsimd.dma_start(m_t[1:127, :], m_in[:])
            m_sh = wpool.tile([127, 256], f32)
            nc.vector.memset(m_sh[:], 0.0)
            nc.gpsimd.dma_start(m_sh[0:126, :], m_in[:])

            # ---------------- basis weights (f32r) ----------------
            # W3w = d_j = (M[j+1]-M[j]) / (6 h_j); W2w = M[j]/2; W1w = b_j
            dm = wpool.tile([127, 256], f32)
            nc.vector.tensor_tensor(dm[:], m_sh[:], m_t[0:127, :], Alu.subtract)
            w3w = cpool.tile([127, 256], f32r)
            nc.vector.tensor_scalar_mul(w3w[:], dm[:], rh6_c[0:127, :])
            w2w = cpool.tile([127, 256], f32r)
            nc.vector.tensor_scalar_mul(w2w[:], m_t[0:127, :], 0.5)
            t1 = wpool.tile([127, 256], f32)
            nc.vector.scalar_tensor_tensor(t1[:], m_t[0:127, :], 2.0, m_sh[:],
                                           Alu.mult, Alu.add)
            w1w = cpool.tile([127, 256], f32r)
            nc.vector.scalar_tensor_tensor(w1w[:], t1[:], hneg6_c[0:127, :], s_sl[:],
                                           Alu.mult, Alu.add)

            # Z-matmul weights (fp32, exact): [ones; -kn]
            negkn = cpool.tile([1, 128], f32)
            nc.vector.tensor_scalar_mul(negkn[:], knr[:], -1.0)
            knw = cpool.tile([2, 128], f32)
            nc.vector.memset(knw[:], 1.0)
            nc.gpsimd.dma_start(knw[1:2, :], negkn[:])

            # ---------------- evaluation ----------------
            n_chunks = PTS // CHUNK
            for ci in range(n_chunks):
                n0 = ci * CHUNK
                zp = zpsum.tile([128, CHUNK], f32)
                nc.tensor.matmul(zp[:], knw[:], xr[:, n0:n0 + CHUNK],
                                 start=True, stop=True)
                u_t = spool.tile([128, CHUNK], f32, tag="u")
                nc.scalar.activation(u_t[:], zp[:], Act.Relu)
                uc = spool.tile([128, CHUNK], f32r, tag="uc")
                nc.vector.tensor_scalar(uc[:], u_t[:], caps_c[:], None, Alu.min)
                s_t = spool.tile([128, CHUNK], f32r, tag="s")
                nc.vector.tensor_tensor(s_t[:], uc[:], uc[:], Alu.mult)
                p_t = spool.tile([128, CHUNK], f32r, tag="p")
                nc.vector.tensor_tensor(p_t[:], uc[:], s_t[:], Alu.mult)
                for half in range(2):
                    cs = slice(half * 128, (half + 1) * 128)
                    a0bias = outT[:, half:half + 1]
                    vp = vpsum.tile([128, CHUNK], f32)
                    nc.tensor.matmul(vp[:], w3w[:, cs], p_t[0:127, :],
                                     start=True, stop=False)
                    nc.tensor.matmul(vp[:], w2w[:, cs], s_t[0:127, :],
                                     start=False, stop=False)
                    nc.tensor.matmul(vp[:], w1w[:, cs], uc[0:127, :],
                                     start=False, stop=True)
                    ob = opool.tile([128, CHUNK], f32, tag="ob")
                    nc.scalar.activation(ob[:], vp[:], Act.Identity, bias=a0bias)
                    dma_eng = (nc.sync, nc.gpsimd)[(ci + half) % 2]
                    dma_eng.dma_start(out_d[cs, 2 * ci:2 * ci + 2, :], ob[:])
    nc.compile()
    return nc


def kernel(**inputs):
    from concourse.bass_utils import run_bass_kernel_spmd

    if "nc" not in _CACHE:
        _CACHE["nc"] = _build_program()
    nc = _CACHE["nc"]

    grid = np.ascontiguousarray(inputs["grid"], dtype=np.float32)
    common = {k: np.ascontiguousarray(np.asarray(v), dtype=np.float32)
              for k, v in inputs.items() if k != "grid"}
    in_maps = []
    for c in range(N_CORES):
        m = dict(common)
        m["gslice"] = np.ascontiguousarray(
            grid[c * ROWS_PER_CORE:(c + 1) * ROWS_PER_CORE])
        in_maps.append(m)
    res = run_bass_kernel_spmd(nc, in_maps, list(range(N_CORES)),
                               trace=bool(_CACHE.get("trace", False)),
                               tmpdir=_CACHE.get("tmpdir"))
    _CACHE["last_res"] = res
    out = np.concatenate([res.results[c]["out"] for c in range(N_CORES)], axis=1)
    return out
